# revision 4
# baseline (speedup 1.0000x reference)
"""Trainium2 Bass kernel for nn_CTRPredictor (gnn_message_passing), v3.

score[e] = dot(normalize(x[src[e]]), normalize(x[dst[e]]))  for E edges.

Strategy (8 NeuronCores, SPMD):
  - Cores 2a, 2a+1 split the edges whose src lies in pair-bank a
    (nodes [25000a, 25000(a+1))), so each core's src rows come from one
    6.4MB bf16 table that fits in SBUF.
  - The src side is "gathered" by the idle TensorEngine: edges are packed
    into 128-slot batches whose src rows fit a 3-block window of a shared
    (core-uniform) block schedule; 3 accumulating one-hot matmuls per batch
    select the rows from the SBUF table (no DMA descriptors at all).
  - The dst side remains a SWDGE dma_gather from the DRAM-replicated table
    (int16 ids within 25088-row apertures), now the only descriptor load on
    the 4 queues (half of the baseline's).
  - Normalized x travels in a row-of-block-on-partition transposed layout:
    a pair AllGather yields the SBUF src table, two half AllGathers build
    the dst apertures, each overlapping gathers on earlier banks.
  - DVE builds one-hots (is_equal vs iota) and does the bf16 mult+reduce;
    ACT drains PSUM to SBUF.
"""

import numpy as np

N = 100000
D = 128
E = 640000
CORES = 8
SLICE = 12500             # nodes normalized per core
PBANK = 25000             # nodes per src pair-bank (2 slices)
SBLK = 98                 # 128-row vblocks per slice (98*128 = 12544)
SLOTPAD = SBLK * 128      # padded slice rows (44 pad rows of ones)
NVB = 2 * SBLK            # vblocks per pair-bank table
HBLK = 49                 # vblocks per half AllGather
APER = 4 * 128 * HBLK     # rows per dst aperture (25088, int16-safe)
NDG = 4                   # dst groups: (half h, rank-half rh)
GCALL = 2560              # slots per dma_gather call (20 batches)
WIN = 3                   # src block window per batch

_CACHE = {}
LAST_RESULTS = None
RUN_KWARGS = {}


def _call_caps(cap):
    calls = []
    while cap > 0:
        c = min(GCALL, cap)
        calls.append(c)
        cap -= c
    return calls


def _build(scheds):
    """scheds: tuple of 4 tuples, per-dst-group batch base-block schedule."""
    from concourse import bass, bacc, tile, mybir

    f32 = mybir.dt.float32
    bf16 = mybir.dt.bfloat16
    i16 = mybir.dt.int16

    slots_dg = [len(s) * 128 for s in scheds]
    slots_total = sum(slots_dg)
    icols_total = slots_total // 16
    scol_total = slots_total // 128

    nc = bacc.Bacc("TRN2", target_bir_lowering=False, debug=False,
                   num_devices=CORES, num_swdge_queues=4,
                   dynamic_dma_scratch_size=40960)

    xsl_d = nc.dram_tensor("xsl", [128, SLOTPAD], f32, kind="ExternalInput")
    ids_d = nc.dram_tensor("ids", [128, slots_total], i16,
                           kind="ExternalInput")
    didx_d = nc.dram_tensor("dst_idx", [128, icols_total], i16,
                            kind="ExternalInput")
    out_d = nc.dram_tensor("out", [128, scol_total], f32,
                           kind="ExternalOutput")

    with tile.TileContext(nc) as tc:
        with tc.tile_pool(name="dram", bufs=1, space="DRAM") as dp, \
             tc.tile_pool(name="persist", bufs=1) as pp:

            didx = pp.tile([128, icols_total], i16)
            nc.sync.dma_start(out=didx[:, :], in_=didx_d.ap())
            score = pp.tile([128, scol_total], f32)
            tbl = pp.tile([128, NVB * D], bf16)
            iot = pp.tile([128, WIN], i16)
            nc.gpsimd.iota(out=iot[:, :], pattern=[[128, WIN]], base=0,
                           channel_multiplier=1)

            # ---- normalize this core's slice (transposed vblock layout) --
            ags = []
            with tc.tile_pool(name="ph0", bufs=1) as p0, \
                 tc.tile_pool(name="sqp", bufs=2) as sqp:
                xsl = p0.tile([128, SLOTPAD], f32)
                ntile = p0.tile([128, SLOTPAD], bf16)
                ns = p0.tile([128, SBLK], f32)
                nrm = p0.tile([128, SBLK], f32)
                rns = p0.tile([128, SBLK], f32)
                bounds = [0, 25, HBLK, 74, SBLK]
                for ci in range(4):
                    b0, b1 = bounds[ci], bounds[ci + 1]
                    csl = slice(b0 * D, b1 * D)
                    nsl = slice(b0, b1)
                    nb = b1 - b0
                    nc.sync.dma_start(out=xsl[:, csl],
                                      in_=xsl_d.ap()[:, csl])
                    sq = sqp.tile([128, 25 * D], f32, tag="sq")
                    nc.scalar.activation(
                        out=sq[:, :nb * D], in_=xsl[:, csl],
                        func=mybir.ActivationFunctionType.Square)
                    nc.vector.tensor_reduce(
                        out=ns[:, nsl],
                        in_=sq[:, :nb * D].rearrange("p (r d) -> p r d", d=D),
                        axis=mybir.AxisListType.X,
                        op=mybir.AluOpType.add,
                    )
                    nc.scalar.activation(
                        out=nrm[:, nsl], in_=ns[:, nsl],
                        func=mybir.ActivationFunctionType.Sqrt)
                    nc.vector.reciprocal(out=rns[:, nsl], in_=nrm[:, nsl])
                    nc.vector.tensor_mul(
                        out=ntile[:, csl].rearrange("p (r d) -> p r d", d=D),
                        in0=xsl[:, csl].rearrange("p (r d) -> p r d", d=D),
                        in1=rns[:, nsl].unsqueeze(-1).to_broadcast(
                            [128, nb, D]),
                    )
                    if ci == 1:   # blocks [0, 49) done -> first half AG
                        agin = dp.tile([128, HBLK * D], bf16, name="agh0")
                        htab = dp.tile([1024, HBLK * D], bf16, name="htab0",
                                       addr_space="Shared")
                        nc.sync.dma_start(out=agin[:, :],
                                          in_=ntile[:, :HBLK * D])
                        nc.gpsimd.collective_compute(
                            "AllGather", mybir.AluOpType.bypass,
                            replica_groups=[list(range(CORES))],
                            ins=[agin.opt()], outs=[htab.opt()],
                        )
                        ags.append(htab)
                    if ci == 3:
                        # pair AG: SBUF src table for cores 2a, 2a+1
                        aginp = dp.tile([128, SLOTPAD], bf16, name="agp")
                        ptab = dp.tile([256, SLOTPAD], bf16, name="pairtab")
                        nc.sync.dma_start(out=aginp[:, :], in_=ntile[:, :])
                        nc.gpsimd.collective_compute(
                            "AllGather", mybir.AluOpType.bypass,
                            replica_groups=[[2 * a, 2 * a + 1]
                                            for a in range(4)],
                            ins=[aginp.opt()], outs=[ptab.opt()],
                        )
                        # second half AG
                        agin1 = dp.tile([128, HBLK * D], bf16, name="agh1")
                        htab1 = dp.tile([1024, HBLK * D], bf16, name="htab1",
                                        addr_space="Shared")
                        nc.sync.dma_start(out=agin1[:, :],
                                          in_=ntile[:, HBLK * D:])
                        nc.gpsimd.collective_compute(
                            "AllGather", mybir.AluOpType.bypass,
                            replica_groups=[list(range(CORES))],
                            ins=[agin1.opt()], outs=[htab1.opt()],
                        )
                        ags.append(htab1)
                        nc.sync.dma_start(out=tbl[:, :SLOTPAD],
                                          in_=ptab[0:128, :])
                        nc.sync.dma_start(out=tbl[:, SLOTPAD:],
                                          in_=ptab[128:256, :])

            # ---- main loop over dst groups / calls ----
            with tc.tile_pool(name="xdp", bufs=4) as xdp, \
                 tc.tile_pool(name="xsp", bufs=3) as xsp, \
                 tc.tile_pool(name="ohp", bufs=2) as ohp, \
                 tc.tile_pool(name="idp", bufs=3) as idp, \
                 tc.tile_pool(name="psp", bufs=6, space="PSUM") as psp:
                qn = 0
                icol_off = 0
                scol_off = 0
                slot_off = 0
                for dg in range(NDG):
                    h, rh = dg // 2, dg % 2
                    aper = ags[h][:, :].rearrange(
                        "q (j f) -> (q j) f", f=D)[rh * APER:(rh + 1) * APER,
                                                   :]
                    sched = scheds[dg]
                    t_base = 0
                    for cap in _call_caps(slots_dg[dg]):
                        cc = cap // 128
                        ic = cap // 16
                        xd = xdp.tile([128, GCALL], bf16, tag="xd")
                        nc.gpsimd.dma_gather(
                            out_ap=xd[:, :cap].rearrange(
                                "p (c d) -> p c d", d=D),
                            in_ap=aper,
                            idxs_ap=didx[:, icol_off:icol_off + ic],
                            num_idxs=cap, num_idxs_reg=cap, elem_size=D,
                            single_packet=False, queue_num=qn % 4,
                        )
                        qn += 1
                        ids_t = idp.tile([128, GCALL], i16, tag="ids")
                        nc.scalar.dma_start(
                            out=ids_t[:, :cap],
                            in_=ids_d.ap()[:, slot_off:slot_off + cap])
                        ohs = []
                        for k in range(WIN):
                            oh = ohp.tile([128, GCALL], bf16, tag=f"oh{k}")
                            nc.vector.tensor_tensor(
                                out=oh[:, :cap],
                                in0=ids_t[:, :cap],
                                in1=iot[:, k:k + 1].to_broadcast([128, cap]),
                                op=mybir.AluOpType.is_equal,
                            )
                            ohs.append(oh)
                        xs = xsp.tile([128, GCALL], bf16, tag="xs")
                        for q in range(cap // 512):
                            ps = psp.tile([128, 512], f32, tag="ps")
                            for b in range(4):
                                t = t_base + q * 4 + b
                                j0 = sched[t]
                                col = (q * 4 + b) * 128
                                for k in range(WIN):
                                    nc.tensor.matmul(
                                        out=ps[:, b * 128:(b + 1) * 128],
                                        lhsT=ohs[k][:, col:col + 128],
                                        rhs=tbl[:, (j0 + k) * D:
                                                (j0 + k + 1) * D],
                                        start=(k == 0), stop=(k == WIN - 1),
                                    )
                            nc.scalar.activation(
                                out=xs[:, q * 512:(q + 1) * 512],
                                in_=ps[:, :],
                                func=mybir.ActivationFunctionType.Copy)
                        nc.vector.tensor_mul(out=xs[:, :cap],
                                             in0=xs[:, :cap],
                                             in1=xd[:, :cap])
                        nc.vector.tensor_reduce(
                            out=score[:, scol_off:scol_off + cc],
                            in_=xs[:, :cap].rearrange("p (c d) -> p c d",
                                                      d=D),
                            axis=mybir.AxisListType.X,
                            op=mybir.AluOpType.add,
                        )
                        icol_off += ic
                        scol_off += cc
                        slot_off += cap
                        t_base += cc

                nc.sync.dma_start(out=out_d.ap(), in_=score[:, :])

    nc.compile()
    return nc


def _src_map(s):
    """src node -> (pair-bank, vblock within table, row within block)."""
    a = s // PBANK
    w = s - a * PBANK
    rank = w // SLICE
    ws = w - rank * SLICE
    return a, SBLK * rank + ws // 128, ws % 128


def _dst_map(n):
    """dst node -> (dst group, aperture-local index)."""
    r = n // SLICE
    w = n - r * SLICE
    j = w // 128
    p = w % 128
    h = (j >= HBLK).astype(np.int64)
    rh = (r >= 4).astype(np.int64)
    local = ((r % 4) * 128 + p) * HBLK + (j - HBLK * h)
    return 2 * h + rh, local


def _wrap_idx(flat):
    blk = flat.reshape(-1, 16).T
    return np.tile(blk, (8, 1))


def _build_scheds(counts, inflate=0, use_mean=False):
    """counts: [8, NDG, NVB] per-core block counts -> uniform schedules."""
    if use_mean:
        caps = np.ceil(counts.mean(axis=0) + 1.0).astype(np.int64) + inflate
    else:
        caps = counts.max(axis=0) + inflate  # [NDG, NVB]
    scheds = []
    for dg in range(NDG):
        rem = caps[dg].astype(np.int64).copy()
        sched = []
        total = int(rem.sum())
        j = 0
        while total > 0:
            while j < NVB - 1 and rem[j] == 0:
                j += 1
            j0 = min(j, NVB - WIN)
            room = 128
            jj = j0
            while room > 0 and jj <= min(j0 + WIN - 1, NVB - 1):
                take = int(min(room, rem[jj]))
                rem[jj] -= take
                room -= take
                total -= take
                if rem[jj] == 0:
                    jj += 1
            sched.append(j0)
        while len(sched) % 4:
            sched.append(NVB - WIN)
        scheds.append(tuple(sched))
    return tuple(scheds)


def _assign_core(edges, scheds):
    """edges: list per dg of (eid, j, p, dl) arrays. Returns slot data."""
    slots_dg = [len(s) * 128 for s in scheds]
    slots_total = sum(slots_dg)
    ids_flat = np.full(slots_total, -1, dtype=np.int16)
    didx_flat = np.zeros(slots_total, dtype=np.int16)
    rows = np.empty(E // CORES + 4096, dtype=np.int64)
    cols = np.empty(E // CORES + 4096, dtype=np.int64)
    n_edges = 0
    slot_off = 0
    t_off = 0
    for dg in range(NDG):
        eid, jb, pr, dl = edges[dg]
        sched = np.asarray(scheds[dg])
        order = np.argsort(jb, kind="stable")
        eid, jb, pr, dl = eid[order], jb[order], pr[order], dl[order]
        # greedy: batches in order take earliest-block remaining edges
        ptr = 0
        nE = eid.size
        for t, j0 in enumerate(sched):
            room = 128
            s0 = slot_off + t * 128
            while room > 0 and ptr < nE and jb[ptr] < j0 + WIN:
                if jb[ptr] < j0:
                    raise RuntimeError("scheduler stranded an edge")
                s = s0 + (128 - room)
                ids_flat[s] = 128 * (jb[ptr] - j0) + pr[ptr]
                didx_flat[s] = dl[ptr]
                e = eid[ptr]
                rows[e] = s % 128
                cols[e] = t_off + t
                room -= 1
                ptr += 1
            if ptr >= nE and room == 128:
                pass  # trailing pad batches
        if ptr < nE:
            raise RuntimeError(f"dg{dg}: {nE - ptr} edges unplaced")
        n_edges += nE
        slot_off += slots_dg[dg]
        t_off += slots_dg[dg] // 128
    return ids_flat, didx_flat, rows, cols, n_edges


def kernel(x, src, dst):
    global LAST_RESULTS
    from concourse.bass_utils import run_bass_kernel_spmd

    x32 = np.ascontiguousarray(np.asarray(x, dtype=np.float32))
    src_i = np.asarray(src).astype(np.int64)
    dst_i = np.asarray(dst).astype(np.int64)

    a_all, jb_all, pr_all = _src_map(src_i)
    dg_all, dl_all = _dst_map(dst_i)

    # edge -> core: pair-bank a holds cores 2a, 2a+1; alternate within
    # each (a, dg, block) bucket so per-core block counts stay balanced
    key = (a_all * NDG + dg_all) * NVB + jb_all
    order = np.argsort(key, kind="stable")
    half = np.empty(E, dtype=np.int64)
    sizes = np.bincount(key, minlength=4 * NDG * NVB)
    off = 0
    for kk in range(4 * NDG * NVB):
        ids_b = order[off:off + sizes[kk]]
        off += sizes[kk]
        half[ids_b] = np.arange(ids_b.size) % 2
    core_all = 2 * a_all + half

    # per-core, per-dg edge arrays + block counts
    counts = np.zeros((CORES, NDG, NVB), dtype=np.int64)
    core_edges = []
    eid_local = np.empty(E, dtype=np.int64)
    for c in range(CORES):
        sel = np.where(core_all == c)[0]
        eid_local[sel] = np.arange(sel.size)
        per_dg = []
        for dg in range(NDG):
            m = sel[dg_all[sel] == dg]
            per_dg.append((eid_local[m], jb_all[m], pr_all[m], dl_all[m]))
            counts[c, dg] = np.bincount(jb_all[m], minlength=NVB)
        core_edges.append((sel, per_dg))

    scheds = None
    for use_mean, inflate in ((True, 0), (True, 1), (True, 3),
                              (False, 0), (False, 2), (False, 8),
                              (False, 32)):
        cand = _build_scheds(counts, inflate, use_mean)
        try:
            for c in range(CORES):
                _assign_core(core_edges[c][1], cand)
        except RuntimeError:
            continue
        scheds = cand
        break
    assert scheds is not None, "schedule infeasible even with inflation"

    if scheds not in _CACHE:
        _CACHE[scheds] = _build(scheds)
    nc = _CACHE[scheds]

    slots_dg = [len(s) * 128 for s in scheds]
    slots_total = sum(slots_dg)

    in_maps = []
    inv = []
    for c in range(CORES):
        sel, per_dg = core_edges[c]
        ids_flat, didx_flat, rows, cols, nE = _assign_core(per_dg, scheds)
        assert nE == sel.size
        inv.append((sel, rows, cols))

        # didx wrap per call
        icols_total = slots_total // 16
        didx = np.zeros((128, icols_total), dtype=np.int16)
        icol_off = 0
        pos = 0
        for dg in range(NDG):
            for cap in _call_caps(slots_dg[dg]):
                ic = cap // 16
                didx[:, icol_off:icol_off + ic] = _wrap_idx(
                    didx_flat[pos:pos + cap])
                icol_off += ic
                pos += cap

        # xsl: slice in transposed vblock layout, pad rows = 1.0
        xs = np.ones((SLOTPAD, D), dtype=np.float32)
        xs[:SLICE] = x32[c * SLICE:(c + 1) * SLICE]
        xsl = np.ascontiguousarray(
            xs.reshape(SBLK, 128, D).transpose(1, 0, 2).reshape(
                128, SLOTPAD))

        in_maps.append({
            "xsl": xsl,
            "ids": np.ascontiguousarray(
                np.tile(ids_flat[None, :], (128, 1))),
            "dst_idx": np.ascontiguousarray(didx),
        })

    res = run_bass_kernel_spmd(nc, in_maps, core_ids=list(range(CORES)),
                               **RUN_KWARGS)
    LAST_RESULTS = res

    out = np.empty(E, dtype=np.float32)
    for c in range(CORES):
        tilev = np.asarray(res.results[c]["out"])
        sel, rows, cols = inv[c]
        nE = sel.size
        out[sel] = tilev[rows[:nE], cols[:nE]]
    return out.reshape(E, 1)
